# revision 42
# baseline (speedup 1.0000x reference)
"""Multi-head graph attention (GAT) on 8 TRN2 NeuronCores.

Reference computation (N=4096 nodes, F_in=512, H=8 heads, F_out=64):
    Wh   = einsum('nf,hfo->hno', features, W)
    src  = Wh @ a_src  (per head), dst = Wh @ a_dst
    e    = leaky_relu(src_i + dst_j, 0.2), masked by adjacency
    attn = softmax(e, axis=-1)
    h    = elu(attn @ Wh)  -> concat heads -> [N, H*F_out]

Sharding: head parallelism - core c owns head c entirely. Each core computes
its head's Wh, the full [N, N] masked softmax, and output columns
out[:, 64c:64c+64]; the host gather is a concatenate. No collectives.

Per-core algorithm ("keys on partitions", scores transposed):
    exp(prelu(src_i + dst_j)) = max(exp(z), exp(0.2 z))        [z = s_i + d_j]
                              = F1[j] * E2[i] * max(E4[i], C0[j])
with E4 = exp(0.8 src), E2 = exp(0.2 src), F1 = exp(dst), C0 = exp(-0.8 dst).
The E2[i] factor is constant per query column, so it scales softmax numerator
and denominator identically and CANCELS - dropped entirely. F1[j] rides the
contraction index, so it folds into the matmul weights (including the ones
column that accumulates the denominator):
    acc[o,i] = sum_j (Wh[j,o]*F1[j]) * pm[j,i]
    pm[j,i]  = adj[j,i] * max(E4[i], C0[j])
The F1 factor folds into the per-tile score via the tensor_scalar's second
scalar slot - v = (e4rep max C0[j]) * F1[j] in ONE 4x DVE op - so the matmul
stationary is just [Wh_j | dst | ones] (whd_all, 66-wide blocks; the ones
column accumulates the softmax denominator, the dst column is a junk row).
Mask-mult split by HW-measured rates: 22 tiles on DVE (2x bf16
tensor_tensor, bf16 adjacency, processed in PAIRS - one 2MB DMA + one
[128, 8192] tt per pair to halve instr/DMA dispatch overhead), 10 on Pool
(fp8 adjacency, two 2048-halves so the PE starts consuming early). The 10/22
split is a sharp HW optimum (8 or 12 Pool tiles are each 15-45us slower -
DVE 2-port ops and Pool share SBUF ports, so the lanes co-saturate).
Adjacency DMAs: SP queue for j<20, ACT queue for late tiles (HWDGE dma_start
holds the issuing sequencer for the whole transfer, so ACT only gets DMAs
once its phase-1 compute has drained). Phase-1 evacuations batch 4 tiles per
strided copy (group 0 on DVE for the fast path to c0g[0] -> v(0)).
Epilogue: 32 transposes pack 4-per-PSUM-bank; denominators gather via 8
strided reciprocals; normalize lands in one contiguous [P, 2048] buffer so
exp / (min,add) / max run as 2 wide ops per half and the output leaves as
TWO 0.5MB DMAs (was 32 small ones). elu(y) = max(y, min(exp(y),1) - 1).
HW-measured traps: fp8-adjacency-with-ACT-upcast loses ~115us (extra
DMA->ACT->DVE hop serializes); DMA-accum (cce) supports only `add` on HW.
"""
import numpy as np
import ml_dtypes

import concourse.bass as bass
import concourse.bacc as bacc
import concourse.tile as tile
import concourse.mybir as mybir
from concourse.bass_utils import run_bass_kernel_spmd

FP32 = mybir.dt.float32
BF16 = mybir.dt.bfloat16
FP8 = mybir.dt.float8e4
AF = mybir.ActivationFunctionType
ALU = mybir.AluOpType
AX = mybir.AxisListType

P = 128          # SBUF partitions
N = 4096         # nodes
F = 512          # input features
H = 8            # heads
FO = 64          # out features per head
C = 8            # cores (1 head each)
JT = N // P      # key tiles = 32
FC = F // P      # feature chunks = 4
QC = N // 512    # query column chunks of 512 = 8
ALPHA = 0.2

# --- tuning knobs (HW-microbenched: DVE ts 4x=1.12us, DVE tt bf16 2x=2.21us,
# Pool tt=7.7us at gpsimd eff 0.42, DVE stt 1x=4.35us, ACT pass=3.5us) ---
# Every tile's max-job (v = e4rep max C0) is a 4x tensor_scalar on DVE.
# The mask-mult splits: MULT_DVE tiles do a 2x bf16 tensor_tensor on DVE
# (bf16 adjacency); the rest multiply on Pool (fp8 adjacency - Pool tt is
# dtype-agnostic, and gpsimd supports only tensor_tensor/copy on HW).
POOL_TILES = (0, 4, 8, 12, 16, 19, 22, 25, 27, 29)
MULT_DVE = tuple(j for j in range(32) if j not in POOL_TILES)
DVE_SLOT = {j: k for k, j in enumerate(MULT_DVE)}
# consecutive DVE tiles are processed as PAIRS: one 2MB adjacency DMA +
# one [128, 2N] tensor_tensor per pair (halves DVE instr count + DMA count,
# each of which costs fixed dispatch/semaphore overhead on HW)
DVE_PAIRS = tuple((MULT_DVE[2 * i], MULT_DVE[2 * i + 1])
                  for i in range(len(MULT_DVE) // 2))
PAIR_OF = {}
for _k, (_a, _b) in enumerate(DVE_PAIRS):
    PAIR_OF[_a] = (_k, 0)
    PAIR_OF[_b] = (_k, 1)



def build_nc(iters=1, loop_n=None, upto=3):
    nc = bacc.Bacc("TRN2", target_bir_lowering=False, debug=False)

    d_ft = nc.dram_tensor("featT", [F, N], BF16, kind="ExternalInput")
    n_dve = len(MULT_DVE)
    # uint8 at the XLA boundary (fp8 unsupported there on TRN2); the DMA
    # bitcasts to fp8e4m3 - same bytes
    d_adj8 = nc.dram_tensor("adjT8", [(32 - n_dve) * P, N], mybir.dt.uint8,
                            kind="ExternalInput")
    # pair-interleaved: block k row p = [adjT[j1*128+p, :], adjT[j2*128+p, :]]
    d_adjb = nc.dram_tensor("adjTb", [len(DVE_PAIRS) * P, 2 * N], BF16,
                            kind="ExternalInput")
    d_wh = nc.dram_tensor("Wh", [F, FO], BF16, kind="ExternalInput")
    d_ah = nc.dram_tensor("ah", [2, FO], BF16, kind="ExternalInput")
    d_id = nc.dram_tensor("ident", [P, P], FP32, kind="ExternalInput")
    d_out = nc.dram_tensor("out", [N, FO], FP32, kind="ExternalOutput")

    from contextlib import ExitStack, nullcontext

    with tile.TileContext(nc) as tc:
      with (tc.For_i(0, loop_n, 1) if loop_n else nullcontext()):
       for _it in range(iters):
        with ExitStack() as stk:
            keep = stk.enter_context(tc.tile_pool(name="keep", bufs=1))

            # ---- persistent tiles ----
            e4rep = keep.tile([P, N], BF16)        # exp(0.8 src) replicated
            # all 32 stationary blocks, stride W2=66: cols 0-63 Wh_j,
            # col 64 dst-proj (junk row in acc), col 65 ones (denominator)
            W1, W2 = FO + 1, FO + 2
            whd_all = keep.tile([P, JT * W2], BF16)
            sdg = [keep.tile([P, 8], FP32, name=f"sdg{g}", tag=f"sdg{g}")
                   for g in range(4)]              # dst proj per group of 8
            f1g = [keep.tile([P, 8], FP32, name=f"f1g{g}", tag=f"f1g{g}")
                   for g in range(4)]              # exp(dst)
            c0g = [keep.tile([P, 8], FP32, name=f"c0g{g}", tag=f"c0g{g}")
                   for g in range(4)]              # exp(-0.8 dst)
            idn = keep.tile([P, P], FP32)
            ones1 = keep.tile([1, P], BF16)
            ar = keep.tile([1, 2 * FO], BF16)
            arep = keep.tile([P, 2 * FO], BF16)
            wt = keep.tile([P, 2 * FC], BF16)      # col 2c = src, 2c+1 = dst
            ht = keep.tile([FO + 2, N], FP32)      # evacuated accumulator

            nc.scalar.dma_start(idn[:], d_id[:])
            nc.vector.memset(whd_all[:], 1.0)      # preset the ones columns
            nc.scalar.dma_start(ar[:], d_ah.ap().rearrange("(x s) o -> x (s o)", x=1))
            nc.vector.memset(ones1[:], 1.0)

            with ExitStack() as ph1:
                sb1 = ph1.enter_context(tc.tile_pool(name="sb1", bufs=1))
                ps1 = ph1.enter_context(tc.tile_pool(name="ps1", bufs=2, space="PSUM"))

                ft = sb1.tile([P, FC * N], BF16)         # featT, 32KB/part
                whsd = sb1.tile([P, FC * (FO + 1)], BF16)  # [Wh_c | wt_dst_c]
                # small weights first so wt/psr/pwh aren't stuck behind the
                # big feature loads; ft h0 (needed by psr q0-3) on ACT, h1 on
                # SP - both halves stream concurrently and ACT's queue drains
                # by ~6us so ACT compute (exps/copies) isn't blocked behind
                # DMA transfers (HWDGE dma_start occupies the issuing SEQ for
                # the whole transfer).
                nc.scalar.dma_start(
                    whsd[:].rearrange("p (c o) -> p c o", c=FC)[:, :, 0:FO],
                    d_wh.ap().rearrange("(c p) o -> p c o", p=P))
                for c_ in range(FC):
                    nc.scalar.dma_start(
                        ft[:, c_ * N: c_ * N + 2048],
                        d_ft[c_ * P:(c_ + 1) * P, 0:2048])
                for c_ in range(FC):
                    nc.sync.dma_start(
                        ft[:, c_ * N + 2048: c_ * N + 4096],
                        d_ft[c_ * P:(c_ + 1) * P, 2048:4096])

                # broadcast [a_src | a_dst] across partitions (k=1 matmul)
                ps_b = ps1.tile([P, 2 * FO], FP32, tag="pb", bufs=1)
                nc.tensor.matmul(ps_b[:], ones1[:], ar[:], start=True, stop=True)
                nc.vector.tensor_copy(arep[:], ps_b[:])

                # wtilde[f] = sum_o Wh[f, o] * a[o]  (src col -> wt, dst col ->
                # wt and whsd's 65th column per chunk)
                lp = stk.enter_context(
                    nc.allow_low_precision(reason="bf16 projection weights"))
                for c_ in range(FC):
                    tmp = sb1.tile([P, 2 * FO], BF16, tag="wtmp")
                    nc.vector.tensor_tensor(
                        tmp[:, 0:FO],
                        whsd[:, c_ * (FO + 1):c_ * (FO + 1) + FO],
                        arep[:, 0:FO], ALU.mult)
                    nc.vector.tensor_tensor(
                        tmp[:, FO:2 * FO],
                        whsd[:, c_ * (FO + 1):c_ * (FO + 1) + FO],
                        arep[:, FO:2 * FO], ALU.mult)
                    nc.vector.tensor_reduce(wt[:, 2 * c_:2 * c_ + 2],
                                            tmp[:].rearrange("p (s o) -> p s o", s=2),
                                            AX.X, ALU.add)
                    nc.vector.tensor_copy(whsd[:, c_ * (FO + 1) + FO:(c_ + 1) * (FO + 1)],
                                          wt[:, 2 * c_ + 1:2 * c_ + 2])

                # stage-major: all psr matmuls first, then all exps, then
                # all broadcasts - engine queues pipeline independently, so
                # the per-chunk PE<->ACT round-trips disappear.
                er4all = sb1.tile([1, N], BF16)
                psrs = {}

                def emit_psr(q):
                    psr = ps1.tile([1, 512], FP32, name=f"psr{q}",
                                   tag="psr", bufs=2)
                    for c_ in range(FC):
                        nc.tensor.matmul(psr[:], wt[:, 2 * c_:2 * c_ + 1],
                                         ft[:, c_ * N + q * 512:c_ * N + (q + 1) * 512],
                                         start=(c_ == 0), stop=(c_ == FC - 1))
                    psrs[q] = psr

                def emit_exp(q):
                    nc.scalar.activation(er4all[:, q * 512:(q + 1) * 512],
                                         psrs[q][:], AF.Exp, scale=0.8)

                def emit_bcast(q):
                    pb = ps1.tile([P, 512], FP32, tag="pb", bufs=1)
                    nc.tensor.matmul(pb[:], ones1[:],
                                     er4all[:, q * 512:(q + 1) * 512],
                                     start=True, stop=True)
                    # split the evacuation across ACT and DVE so neither
                    # engine's serial chain alone gates e4rep readiness
                    if q < 4:
                        nc.scalar.copy(e4rep[:, q * 512:(q + 1) * 512], pb[:])
                    else:
                        nc.vector.tensor_copy(e4rep[:, q * 512:(q + 1) * 512],
                                              pb[:])

                # Wh computed TRANSPOSED first - WhT[o, n] via 32 wide
                # matmuls whose stationary (whsd chunk) is shared across 4
                # consecutive banks (1 Ldweights per chunk), then 32 per-tile
                # PE transposes restore [keys, 65]. Cuts ~64 matmul + ~90
                # Ldweights dispatches off the PE sequencer vs per-tile pw.
                whtsb = sb1.tile([W1, N], BF16, name="whtsb", tag="whtsb")
                idnb = keep.tile([P, P], BF16, name="idnb", tag="idnb")
                nc.vector.tensor_copy(idnb[:], idn[:])

                def emit_wht(qt):
                    # quarter granularity: WhT keys [1024*qt, 1024*(qt+1))
                    whtps = ps1.tile([W1, 1024], FP32, name=f"whtps{qt}",
                                     tag="whtps", bufs=2)
                    for c_ in range(FC):
                        for b in range(2):
                            col = c_ * N + qt * 1024 + b * 512
                            nc.tensor.matmul(
                                whtps[:, b * 512:(b + 1) * 512],
                                whsd[:, c_ * W1:(c_ + 1) * W1],
                                ft[:, col:col + 512],
                                start=(c_ == 0), stop=(c_ == FC - 1))
                    nc.scalar.copy(whtsb[:, qt * 1024:(qt + 1) * 1024],
                                   whtps[:])

                def emit_transp(blk):
                    j0 = 4 * blk
                    # 66-wide slots: 65*2B slices break PSUM 4B alignment
                    ptp = ps1.tile([P, 4 * W2], BF16, name=f"ptp{blk}",
                                   tag="ptp", bufs=1)
                    for t in range(4):
                        j = j0 + t
                        nc.tensor.transpose(ptp[:, t * W2:t * W2 + W1],
                                            whtsb[:, j * P:(j + 1) * P],
                                            idnb[0:W1, 0:W1])
                    dst = whd_all[:, j0 * W2:(j0 + 4) * W2].rearrange(
                        "p (t c) -> p t c", t=4)[:, :, 0:W1]
                    srcv = ptp[:].rearrange("p (t c) -> p t c", t=4)[:, :, 0:W1]
                    if blk < 2:
                        nc.vector.tensor_copy(dst, srcv)
                    else:
                        nc.scalar.copy(dst, srcv)

                def emit_sdg(g):
                    nc.scalar.copy(
                        sdg[g][:].rearrange("p (t c) -> p t c", c=1),
                        whd_all[:, 8 * g * W2:(8 * g + 8) * W2].rearrange(
                            "p (t c) -> p t c", t=8)[:, :, FO:FO + 1])
                    nc.scalar.activation(f1g[g][:], sdg[g][:], AF.Exp)
                    nc.scalar.activation(c0g[g][:], sdg[g][:], AF.Exp,
                                         scale=-0.8)

                for q in range(4):
                    emit_psr(q)
                emit_wht(0)      # only 8 mms: psr q4-7 wait on ft h1 anyway
                for q in range(4, QC):
                    emit_psr(q)
                for q in range(QC):
                    emit_exp(q)
                for q in range(QC):
                    emit_bcast(q)
                # interleave remaining WhT quarters with their transposes so
                # each group's c0g chain completes as early as possible
                emit_transp(0)
                emit_transp(1)
                emit_sdg(0)       # c0g[0] fast path: only blocks 0-1
                for qt in range(1, 4):
                    emit_wht(qt)
                    emit_transp(2 * qt)
                    emit_transp(2 * qt + 1)
                    emit_sdg(qt)

            if upto < 1:
                with ExitStack() as phx:
                    sbx = phx.enter_context(tc.tile_pool(name="sbx", bufs=1))
                    junk = sbx.tile([P, FO], FP32)
                    nc.vector.memset(junk[:], 0.0)
                    for i in range(JT):
                        nc.sync.dma_start(d_out[i * P:(i + 1) * P, :], junk[:])
                continue
            # ---- phase 2: fused masked-exp scores + V-matmul ----
            # DMA queue policy: SP (sync) is the main adjacency stream; the
            # ACT (scalar) queue takes only the LATE tiles (j >= ACT_ADJ_J) -
            # emitted after all phase-1 ACT compute, so they never block the
            # exps/copies, and they relieve the SP queue's serial backlog.
            ACT_ADJ_J = 17
            with ExitStack() as ph2:
                sb2 = ph2.enter_context(tc.tile_pool(name="sb2", bufs=2))
                adjp = ph2.enter_context(tc.tile_pool(name="adjp", bufs=4))
                acc_pool = ph2.enter_context(
                    tc.tile_pool(name="accps", bufs=1, space="PSUM"))
                acc = acc_pool.tile([FO + 2, N], FP32)   # all 8 banks

                pool_slot = 0
                HN = N // 2

                def emit_mms(j, halves):
                    for q in range(QC):
                        h = halves[q // 4]
                        nc.tensor.matmul(acc[:, q * 512:(q + 1) * 512],
                                         whd_all[:, j * W2:(j + 1) * W2],
                                         h[:, (q % 4) * 512:(q % 4 + 1) * 512],
                                         start=(j == 0), stop=(j == JT - 1))

                pair_state = {}
                for j in range(JT):
                    g, s = j // 8, j % 8
                    if j in PAIR_OF:
                        k, half = PAIR_OF[j]
                        if half == 0:
                            at2 = adjp.tile([P, 2 * N], BF16, tag="atb",
                                            bufs=4)
                            dq = nc.scalar if j >= ACT_ADJ_J else nc.sync
                            dq.dma_start(at2[:],
                                         d_adjb[k * P:(k + 1) * P, :])
                            v2 = sb2.tile([P, 2 * N], BF16, tag="v2", bufs=3)
                            nc.vector.tensor_scalar(v2[:, 0:N], e4rep[:],
                                                    c0g[g][:, s:s + 1],
                                                    f1g[g][:, s:s + 1],
                                                    ALU.max, ALU.mult)
                            pair_state[k] = (at2, v2)
                        else:
                            at2, v2 = pair_state.pop(k)
                            nc.vector.tensor_scalar(v2[:, N:2 * N], e4rep[:],
                                                    c0g[g][:, s:s + 1],
                                                    f1g[g][:, s:s + 1],
                                                    ALU.max, ALU.mult)
                            # in-place: adjacency is dead after the mask-mult,
                            # so the product overwrites at2 (elementwise
                            # streaming, same offsets - safe on DVE)
                            nc.vector.tensor_tensor(at2[:], at2[:], v2[:],
                                                    ALU.mult)
                            j1 = DVE_PAIRS[k][0]
                            emit_mms(j1, [at2[:, 0:HN], at2[:, HN:N]])
                            emit_mms(j, [at2[:, N:N + HN], at2[:, N + HN:2 * N]])
                    else:
                        v = sb2.tile([P, N], BF16, tag="v", bufs=2)
                        nc.vector.tensor_scalar(v[:], e4rep[:],
                                                c0g[g][:, s:s + 1],
                                                f1g[g][:, s:s + 1],
                                                ALU.max, ALU.mult)
                        k = pool_slot
                        pool_slot += 1
                        at = adjp.tile([P, N], FP8, tag="at", bufs=4)
                        nc.sync.dma_start(at[:], d_adj8[k * P:(k + 1) * P, :].bitcast(FP8))
                        pma = sb2.tile([P, HN], BF16, tag="pma", bufs=3)
                        pmb = sb2.tile([P, HN], BF16, tag="pmb", bufs=3)
                        nc.gpsimd.tensor_tensor(pma[:], at[:, 0:HN],
                                                v[:, 0:HN], ALU.mult)
                        nc.gpsimd.tensor_tensor(pmb[:], at[:, HN:N],
                                                v[:, HN:N], ALU.mult)
                        emit_mms(j, [pma[:], pmb[:]])

                for q in range(8):
                    nc.scalar.copy(ht[:, q * 512:(q + 1) * 512],
                                   acc[:, q * 512:(q + 1) * 512])

            if upto < 2:
                with ExitStack() as phx:
                    sbx = phx.enter_context(tc.tile_pool(name="sbx", bufs=1))
                    junk = sbx.tile([P, FO], FP32)
                    nc.vector.memset(junk[:], 0.0)
                    for i in range(JT):
                        nc.sync.dma_start(d_out[i * P:(i + 1) * P, :], junk[:])
                continue
            # ---- epilogue: transpose, normalize, ELU ----
            # Batched: y[i] = tp_i * rc_i lands in ONE contiguous [P, 32*64]
            # buffer, so exp / (min,add) / max each run as a single wide op,
            # and the 32 output writes collapse into ONE 1MB DMA
            # (d_out viewed as [p, (tile, o)]). elu(y) = max(y, min(e^y,1)-1).
            with ExitStack() as ph3:
                ps3 = ph3.enter_context(tc.tile_pool(name="ps3", bufs=1, space="PSUM"))
                sb3 = ph3.enter_context(tc.tile_pool(name="sb3", bufs=1))
                tp4 = [ps3.tile([P, 4 * W2], FP32, name=f"tp4{b}",
                                tag=f"tp4{b}") for b in range(8)]
                rc = sb3.tile([P, JT], FP32)
                yb = sb3.tile([P, JT * FO], FP32)
                eyb = sb3.tile([P, JT * FO], FP32)
                emb = sb3.tile([P, JT * FO], FP32)
                osb = sb3.tile([P, JT * FO], FP32)

                def tpv(i):
                    b, m = i // 4, i % 4
                    return tp4[b][:, m * W2:(m + 1) * W2]

                for i in range(JT):
                    nc.tensor.transpose(tpv(i), ht[:, i * P:(i + 1) * P],
                                        idn[0:W2, 0:W2])
                # four quarter-pipelines: recip -> normalize -> exp ->
                # (min,add) -> max -> DMA per 8 tiles, so the first output
                # DMA fires while later banks are still transposing
                QT4 = JT // 4
                for h in range(4):
                    for b in (2 * h, 2 * h + 1):
                        nc.vector.reciprocal(
                            rc[:, 4 * b:4 * b + 4].rearrange(
                                "p (m o) -> p m o", o=1),
                            tp4[b][:].rearrange(
                                "p (m o) -> p m o", m=4)[:, :, W1:W1 + 1])
                    for i in range(h * QT4, (h + 1) * QT4):
                        # Prelu(alpha=1) == identity: yb = tp * rc on ACT
                        # (idle at the tail) instead of the just-drained DVE
                        nc.scalar.activation(yb[:, i * FO:(i + 1) * FO],
                                             tpv(i)[:, 0:FO], AF.Prelu,
                                             scale=rc[:, i:i + 1], alpha=1.0)
                    sl = slice(h * QT4 * FO, (h + 1) * QT4 * FO)
                    nc.scalar.activation(eyb[:, sl], yb[:, sl], AF.Exp)
                    nc.vector.tensor_scalar(emb[:, sl], eyb[:, sl], 1.0, -1.0,
                                            ALU.min, ALU.add)
                    nc.vector.tensor_tensor(osb[:, sl], yb[:, sl],
                                            emb[:, sl], ALU.max)
                    nc.sync.dma_start(
                        d_out.ap().rearrange("(t p) o -> p t o", p=P)
                        [:, h * QT4:(h + 1) * QT4, :],
                        osb[:, sl].rearrange("p (t o) -> p t o", t=QT4))

    nc.compile()
    return nc


_NC_CACHE = None


def get_nc():
    global _NC_CACHE
    if _NC_CACHE is None:
        _NC_CACHE = build_nc()
    return _NC_CACHE


def make_in_maps(features, adjacency_matrix, W, a_src, a_dst):
    featT = np.ascontiguousarray(features.T).astype(ml_dtypes.bfloat16)
    adjT = np.ascontiguousarray(adjacency_matrix.T)
    pool_tiles = [j for j in range(JT) if j not in DVE_SLOT]
    pool_rows = np.concatenate(
        [np.arange(j * P, (j + 1) * P) for j in pool_tiles])
    adjT8 = np.ascontiguousarray(adjT[pool_rows]).astype(
        ml_dtypes.float8_e4m3fn).view(np.uint8)
    # pair-interleaved bf16 blocks: block k, row p = [adjT[j1*P+p] | adjT[j2*P+p]]
    blocks = [np.concatenate([adjT[j1 * P:(j1 + 1) * P],
                              adjT[j2 * P:(j2 + 1) * P]], axis=1)
              for (j1, j2) in DVE_PAIRS]
    adjTb = np.concatenate(blocks, axis=0).astype(ml_dtypes.bfloat16)
    ident = np.eye(P, dtype=np.float32)
    in_maps = []
    for h in range(C):
        in_maps.append({
            "featT": featT,
            "adjT8": adjT8,
            "adjTb": adjTb,
            "Wh": np.ascontiguousarray(W[h]).astype(ml_dtypes.bfloat16),
            "ah": np.ascontiguousarray(
                np.stack([a_src[h], a_dst[h]])).astype(ml_dtypes.bfloat16),
            "ident": ident,
        })
    return in_maps


def kernel(features, adjacency_matrix, W, a_src, a_dst, _trace=False, _tmpdir=None):
    nc = get_nc()
    in_maps = make_in_maps(np.asarray(features, dtype=np.float32),
                           np.asarray(adjacency_matrix),
                           np.asarray(W, dtype=np.float32),
                           np.asarray(a_src, dtype=np.float32),
                           np.asarray(a_dst, dtype=np.float32))
    res = run_bass_kernel_spmd(nc, in_maps, list(range(C)),
                               trace=_trace, tmpdir=_tmpdir)
    out = np.concatenate([res.results[h]["out"] for h in range(C)], axis=1)
    if _trace:
        kernel.last_results = res
    return out

